# revision 9
# baseline (speedup 1.0000x reference)
"""Trainium2 Bass kernel for the DeepBSDE loss (nn_BaseDeepBSDE).

Data-parallel over 8 NeuronCores: each core simulates 2048 Monte-Carlo
paths through the 100-step SDE loop and produces a partial loss sum;
the host gathers the 8 partial scalars.

Per core, the 2048 paths are split into TWO independent groups of 1024
(chunks 0-7 and 8-15). Each group carries its own y-state recurrence so
the two per-step serial chains pipeline against each other across all
engines. The Y/loss bookkeeping dangles off the recurrence and runs
merged. Layouts:
  - folded state: [128 partitions, 16] with path b = c*128 + p
  - MLP activations feature-major [128 feat, batch], bf16 matmuls
  - y state as [8, 128] row-chunks per group (K=8 block matmuls)
  - noise pre-folded on host to [128, steps*48]; no per-step DMA
  - no fp32 matmuls inside the step loop
"""

import os
import sys

sys.path.insert(0, "/opt/trn_rl_repo")

import numpy as np

B = 16384
NSTEPS = 100
DIMW = 3
DT = 0.01
SQRT_DT = DT**0.5
SIGMA0 = 0.5
NCORES = 8
BC = B // NCORES  # 2048 paths per core
NCH = BC // 128  # 16 chunks of 128 paths
NG = 2  # independent path groups per core
GCH = NCH // NG  # 8 chunks per group
NQ = 4  # noise quarter-buffers

LAST_EXEC_NS = None
LAST_RESULTS = None

_CACHE = {}


def _build(nsteps, debug=False):
    import concourse.tile as tile
    from concourse import bacc, mybir

    f32 = mybir.dt.float32
    bf16 = mybir.dt.bfloat16
    AF = mybir.ActivationFunctionType
    ALU = mybir.AluOpType
    AX = mybir.AxisListType

    nc = bacc.Bacc("TRN2", target_bir_lowering=False, debug=False, num_devices=NCORES)

    # ---------------- DRAM I/O ----------------
    QSTEPS = (nsteps + NQ - 1) // NQ
    dWf_d = [
        nc.dram_tensor(f"dWf{q}", [128, QSTEPS * 48], f32, kind="ExternalInput").ap()
        for q in range(NQ)
    ]
    dZf_d = [
        nc.dram_tensor(f"dZf{q}", [128, QSTEPS * 48], f32, kind="ExternalInput").ap()
        for q in range(NQ)
    ]
    L1b_d = [
        nc.dram_tensor(f"L1b{g}", [GCH, GCH * 128], f32, kind="ExternalInput").ap()
        for g in range(NG)
    ]
    W1cT_d = nc.dram_tensor("W1cT", [128, 2], f32, kind="ExternalInput").ap()
    W2bd_d = nc.dram_tensor("W2bd", [128, 128], f32, kind="ExternalInput").ap()
    W3c_d = nc.dram_tensor("W3c", [128, 4], f32, kind="ExternalInput").ap()
    b1c_d = nc.dram_tensor("b1c", [128, 1], f32, kind="ExternalInput").ap()
    b2c_d = nc.dram_tensor("b2c", [128, 1], f32, kind="ExternalInput").ap()
    b3c_d = nc.dram_tensor("b3c", [1, 4], f32, kind="ExternalInput").ap()
    trep_d = nc.dram_tensor("trep", [128, nsteps], f32, kind="ExternalInput").ap()
    ones_col_d = nc.dram_tensor("ones_col", [128, 1], f32, kind="ExternalInput").ap()
    ones_row_d = nc.dram_tensor("ones_row", [1, 128], f32, kind="ExternalInput").ap()
    I128_d = nc.dram_tensor("I128", [128, 128], f32, kind="ExternalInput").ap()
    y_init_d = nc.dram_tensor("y_init", [GCH * NG, 128], f32, kind="ExternalInput").ap()
    Y_init_d = nc.dram_tensor("Y_init", [128, 16], f32, kind="ExternalInput").ap()

    loss_out = nc.dram_tensor("loss_out", [1, 1], f32, kind="ExternalOutput").ap()
    if debug:
        y_out = nc.dram_tensor("y_out", [16, 128], f32, kind="ExternalOutput").ap()
        Y_out = nc.dram_tensor("Y_out", [128, 16], f32, kind="ExternalOutput").ap()
        zq_out = nc.dram_tensor("zq_out", [128, 64], f32, kind="ExternalOutput").ap()

    with tile.TileContext(nc) as tc:
        from contextlib import ExitStack

        with ExitStack() as ctx:
            cpool = ctx.enter_context(tc.tile_pool(name="const", bufs=1))
            hpool = ctx.enter_context(tc.tile_pool(name="hsb", bufs=2))
            epool = ctx.enter_context(tc.tile_pool(name="ep", bufs=2))
            pmm = ctx.enter_context(tc.tile_pool(name="pmm", bufs=2, space="PSUM"))
            pzq = ctx.enter_context(tc.tile_pool(name="pzq", bufs=2, space="PSUM"))
            ptr = ctx.enter_context(tc.tile_pool(name="ptr", bufs=1, space="PSUM"))

            # ------------- persistent SBUF tiles -------------
            dWs = [cpool.tile([128, QSTEPS * 48], f32, tag=f"dw{q}", name=f"dws{q}") for q in range(NQ)]
            dZs = [cpool.tile([128, QSTEPS * 48], f32, tag=f"dz{q}", name=f"dzs{q}") for q in range(NQ)]
            swp = cpool.tile([128, nsteps * 16], f32, tag="swp")
            L1b_bf = [cpool.tile([GCH, GCH * 128], bf16, tag=f"l1b{g}", name=f"l1bbf{g}") for g in range(NG)]
            W2bd_bf = cpool.tile([128, 128], bf16, tag="w2bd")
            W3_bf = cpool.tile([128, 4], bf16, tag="w3")
            W3_f = cpool.tile([128, 4], f32, tag="w3f")
            b1tab = cpool.tile([128, nsteps], f32, tag="b1tab")
            W1cT_sb = cpool.tile([128, 2], f32, tag="w1ct")
            trep = cpool.tile([128, nsteps], f32, tag="trep")
            b1c_sb = cpool.tile([128, 1], f32, tag="b1c")
            b2c_sb = cpool.tile([128, 1], f32, tag="b2c")
            b3s = cpool.tile([1, 4], f32, tag="b3s")
            b3f = cpool.tile([1, 4], f32, tag="b3f")
            b3rep = cpool.tile([1, 32], bf16, tag="b3rep")
            b3t32 = cpool.tile([128, 32], f32, tag="b3t32")
            ones_bf = cpool.tile([1, 128], bf16, tag="ones_bf")
            ones_col = cpool.tile([128, 1], f32, tag="ones_col")
            I128 = cpool.tile([128, 128], f32, tag="i128")
            I128_bf = cpool.tile([128, 128], bf16, tag="i128bf")
            y16 = [cpool.tile([GCH, 128], f32, tag=f"y16{g}", name=f"y16{g}") for g in range(NG)]
            ypad = [cpool.tile([GCH, 128], bf16, tag=f"ypad{g}", name=f"ypad{g}") for g in range(NG)]
            Y_f = cpool.tile([128, 16], f32, tag="Yf")
            loss_acc = cpool.tile([128, 16], f32, tag="loss_acc")
            ysq = [cpool.tile([GCH, 128], f32, tag=f"ysq{g}", name=f"ysq{g}") for g in range(NG)]
            ee = cpool.tile([128, 16], f32, tag="ee")
            loss_sb = cpool.tile([1, 16], f32, tag="loss_sb")
            loss1 = cpool.tile([1, 1], f32, tag="loss1")

            # ------------- init: DMAs -------------
            for q in range(NQ):
                nc.sync.dma_start(dWs[q][:], dWf_d[q][:])
                nc.sync.dma_start(dZs[q][:], dZf_d[q][:])
            for g in range(NG):
                nc.gpsimd.dma_start(L1b_bf[g][:], L1b_d[g][:])
            nc.gpsimd.dma_start(W2bd_bf[:], W2bd_d[:])
            nc.gpsimd.dma_start(ones_bf[:], ones_row_d[:])
            nc.gpsimd.dma_start(I128_bf[:], I128_d[:])
            nc.sync.dma_start(W3_f[:], W3c_d[:])
            nc.sync.dma_start(W1cT_sb[:], W1cT_d[:])
            nc.sync.dma_start(trep[:], trep_d[:])
            nc.sync.dma_start(b1c_sb[:], b1c_d[:])
            nc.sync.dma_start(b2c_sb[:], b2c_d[:])
            nc.sync.dma_start(b3f[:], b3c_d[:])
            nc.sync.dma_start(ones_col[:], ones_col_d[:])
            nc.sync.dma_start(I128[:], I128_d[:])
            for g in range(NG):
                nc.sync.dma_start(y16[g][:], y_init_d[g * GCH : (g + 1) * GCH, :])
            nc.sync.dma_start(Y_f[:], Y_init_d[:])

            # ------------- init: compute (no fp32 matmuls) -------------
            nc.vector.tensor_scalar(
                b1tab[:], trep[:], W1cT_sb[:, 0:1], b1c_sb[:, 0:1],
                op0=ALU.mult, op1=ALU.add,
            )
            nc.vector.tensor_scalar_mul(W3_bf[:, 0:3], W3_f[:, 0:3], float(SQRT_DT))
            nc.vector.tensor_scalar_mul(W3_bf[:, 3:4], W3_f[:, 3:4], float(DT))
            nc.vector.tensor_scalar_mul(b3s[0:1, 0:3], b3f[0:1, 0:3], float(SQRT_DT))
            nc.vector.tensor_scalar_mul(b3s[0:1, 3:4], b3f[0:1, 3:4], float(DT))
            nc.vector.tensor_copy(b3rep[0:1, 0:4], b3s[0:1, :])
            nc.vector.tensor_copy(b3rep[0:1, 4:8], b3rep[0:1, 0:4])
            nc.vector.tensor_copy(b3rep[0:1, 8:16], b3rep[0:1, 0:8])
            nc.vector.tensor_copy(b3rep[0:1, 16:32], b3rep[0:1, 0:16])
            # b3t32[p, :] = b3rep (partition broadcast via one bf16 matmul)
            b3ps = pzq.tile([128, 32], f32, tag="zq", name="b3ps")
            nc.tensor.matmul(b3ps[:], ones_bf[0:1, :], b3rep[0:1, :], start=True, stop=True)
            nc.scalar.copy(b3t32[:], b3ps[:])

            nc.vector.memset(loss_acc[:], 0.0)

            # sw prepass
            for q in range(NQ):
                nsq = max(0, min(nsteps, (q + 1) * QSTEPS) - q * QSTEPS)
                if nsq == 0:
                    continue
                lo = q * QSTEPS * 16
                src = dWs[q][:, 0 : nsq * 48].rearrange("p (s j) -> p s j", j=3)
                nc.vector.tensor_reduce(
                    swp[:, lo : lo + nsq * 16], src, axis=AX.X, op=ALU.add
                )
            nc.vector.tensor_scalar_mul(swp[:], swp[:], float(SIGMA0 * SQRT_DT))

            # ------------- time-step loop (software-pipelined emission) ----
            # Each group's recurrence tail (transpose + y update) is emitted
            # at the head of that group's NEXT-step block so the PE queue
            # never waits on the DVE tail at a step boundary.
            SC_F = float((0.5 / DT) ** 0.5)
            pend = [None] * NG  # pending (incr_ap, step) per group

            def emit_tail(g):
                incr_ap, i_prev = pend[g]
                tr = ptr.tile([GCH, 128], bf16, tag=f"tr{g}", name=f"tr{i_prev}g{g}")
                nc.tensor.matmul(tr[:], incr_ap, I128_bf[:], is_transpose=True)
                nc.vector.tensor_tensor(y16[g][:], y16[g][:], tr[:], op=ALU.add)
                pend[g] = None

            for i in range(nsteps):
                qi, ri = divmod(i, QSTEPS)

                zq_sb = [None] * NG
                for g in range(NG):
                    # ---- pending tail from step i-1 ----
                    if pend[g] is not None:
                        emit_tail(g)
                    nc.vector.tensor_copy(ypad[g][:], y16[g][:])

                    # ---- L1 ----
                    h1ps = pmm.tile([128, 1024], f32, tag="mm", name=f"h1ps{i}g{g}")
                    for c in range(GCH):
                        nc.tensor.matmul(
                            h1ps[:, c * 128 : (c + 1) * 128],
                            L1b_bf[g][:, c * 128 : (c + 1) * 128],
                            ypad[g][:],
                            start=True,
                            stop=True,
                        )
                    h1sb = hpool.tile([128, 1024], bf16, tag=f"h1{g}", name=f"h1sb{i}g{g}")
                    if g == 0:
                        nc.scalar.activation(
                            h1sb[:], h1ps[:], AF.Relu, bias=b1tab[:, i : i + 1]
                        )
                    else:
                        nc.vector.tensor_scalar(
                            h1sb[:], h1ps[:], b1tab[:, i : i + 1], 0.0,
                            op0=ALU.add, op1=ALU.max,
                        )
                    # ---- L2 ----
                    h2ps = pmm.tile([128, 1024], f32, tag="mm", name=f"h2ps{i}g{g}")
                    for s in range(2):
                        nc.tensor.matmul(
                            h2ps[:, s * 512 : (s + 1) * 512],
                            W2bd_bf[:],
                            h1sb[:, s * 512 : (s + 1) * 512],
                            start=True,
                            stop=True,
                        )
                    h2sb = hpool.tile([128, 1024], bf16, tag=f"h2{g}", name=f"h2sb{i}g{g}")
                    if g == 0:
                        nc.vector.tensor_scalar(
                            h2sb[:], h2ps[:], b2c_sb[:, 0:1], 0.0,
                            op0=ALU.add, op1=ALU.max,
                        )
                    else:
                        nc.scalar.activation(
                            h2sb[:], h2ps[:], AF.Relu, bias=b2c_sb[:, 0:1]
                        )
                    # ---- L3 transposed ----
                    zq_ps = pzq.tile([128, 32], f32, tag="zq", name=f"zqps{i}g{g}")
                    for c in range(GCH):
                        nc.tensor.matmul(
                            zq_ps[:, c * 4 : (c + 1) * 4],
                            h2sb[:, c * 128 : (c + 1) * 128],
                            W3_bf[:],
                            start=True,
                            stop=True,
                        )
                    z = epool.tile([128, 32], f32, tag=f"zqsb{g}", name=f"zqsb{i}g{g}")
                    zq_sb[g] = z
                    nc.vector.tensor_tensor(z[:], zq_ps[:], b3t32[:], op=ALU.add)
                    # ---- incr (gpsimd); transpose deferred to step i+1 ----
                    qview = z[:].rearrange("p (c m) -> p c m", m=4)[:, :, 3:4]
                    incr = epool.tile([128, GCH], bf16, tag=f"incr{g}", name=f"incr{i}g{g}")
                    nc.gpsimd.tensor_tensor(
                        incr[:].rearrange("p (c o) -> p c o", o=1),
                        qview,
                        swp[:, i * 16 + g * 8 : i * 16 + (g + 1) * 8].rearrange(
                            "p (c o) -> p c o", o=1
                        ),
                        op=ALU.add,
                    )
                    pend[g] = (incr[:], i)

                # ---- merged off-chain epilogue ----
                dwv = dWs[qi][:, ri * 48 : (ri + 1) * 48].rearrange("p (c j) -> p c j", j=3)
                dzv = dZs[qi][:, ri * 48 : (ri + 1) * 48].rearrange("p (c j) -> p c j", j=3)
                zz = epool.tile([128, 96], f32, tag="zz", name=f"zz{i}")
                uv = epool.tile([128, 32], f32, tag="uv", name=f"uv{i}")
                r_t = epool.tile([128, 16], f32, tag="r", name=f"r{i}")
                rr_t = epool.tile([128, 16], f32, tag="rr", name=f"rr{i}")
                fDT = epool.tile([128, 16], f32, tag="fdt", name=f"fdt{i}")
                umf = epool.tile([128, 16], f32, tag="umf", name=f"umf{i}")
                for g in range(NG):
                    zview = zq_sb[g][:].rearrange("p (c m) -> p c m", m=4)[:, :, 0:3]
                    qview = zq_sb[g][:].rearrange("p (c m) -> p c m", m=4)[:, :, 3:4]
                    nc.gpsimd.tensor_tensor(
                        zz[:, g * 24 : (g + 1) * 24].rearrange("p (c j) -> p c j", j=3),
                        zview,
                        dwv[:, g * 8 : (g + 1) * 8, :],
                        op=ALU.mult,
                    )
                    nc.gpsimd.tensor_tensor(
                        zz[:, 48 + g * 24 : 48 + (g + 1) * 24].rearrange(
                            "p (c j) -> p c j", j=3
                        ),
                        zview,
                        dzv[:, g * 8 : (g + 1) * 8, :],
                        op=ALU.mult,
                    )
                    nc.scalar.activation(
                        fDT[:, g * 8 : (g + 1) * 8].rearrange("p (c o) -> p c o", o=1),
                        qview,
                        AF.Square,
                        scale=SC_F,
                    )
                nc.vector.tensor_reduce(
                    uv[:],
                    zz[:].rearrange("p (h j) -> p h j", j=3),
                    axis=AX.X,
                    op=ALU.add,
                )
                nc.gpsimd.tensor_tensor(
                    r_t[:], uv[:, 0:16], uv[:, 16:32], op=ALU.subtract
                )
                nc.scalar.activation(rr_t[:], r_t[:], AF.Square)
                nc.gpsimd.tensor_tensor(loss_acc[:], loss_acc[:], rr_t[:], op=ALU.add)
                nc.gpsimd.tensor_tensor(umf[:], uv[:, 0:16], fDT[:], op=ALU.subtract)
                nc.gpsimd.tensor_tensor(Y_f[:], Y_f[:], umf[:], op=ALU.add)

            # ------------- terminal loss (fp32 matmuls OK here) -------------
            for g in range(NG):
                if pend[g] is not None:
                    emit_tail(g)
            for g in range(NG):
                nc.scalar.activation(ysq[g][:], y16[g][:], AF.Square)
                ysq_ps = pzq.tile([128, GCH], f32, tag="zq", name=f"ysqps{g}")
                nc.tensor.matmul(
                    ysq_ps[:], ysq[g][:], I128[0:GCH, 0:GCH], is_transpose=True
                )
                nc.vector.tensor_tensor(
                    ee[:, g * 8 : (g + 1) * 8],
                    Y_f[:, g * 8 : (g + 1) * 8],
                    ysq_ps[:],
                    op=ALU.subtract,
                )
            nc.scalar.activation(ee[:], ee[:], AF.Square)
            nc.vector.tensor_tensor(loss_acc[:], loss_acc[:], ee[:], op=ALU.add)
            lps = pzq.tile([1, 16], f32, tag="zq", name="lps")
            nc.tensor.matmul(lps[:], ones_col[:], loss_acc[:], start=True, stop=True)
            nc.vector.tensor_copy(loss_sb[:], lps[:])
            nc.vector.tensor_reduce(
                loss1[:],
                loss_sb[0:1, :].rearrange("p (o c) -> p o c", o=1),
                axis=AX.X,
                op=ALU.add,
            )
            nc.vector.tensor_scalar_mul(loss1[:], loss1[:], 1.0 / B)
            nc.sync.dma_start(loss_out[:], loss1[:])
            if debug:
                for g in range(NG):
                    nc.sync.dma_start(y_out[g * GCH : (g + 1) * GCH, :], y16[g][:])
                nc.sync.dma_start(Y_out[:], Y_f[:])
                for g in range(NG):
                    nc.sync.dma_start(
                        zq_out[:, g * 32 : (g + 1) * 32], zq_sb[g][:]
                    )

    nc.compile()
    return nc


def _host_inputs(nsteps, y0, Y0, zW1, zb1, zW2, zb2, zW3, zb3, qW1, qb1, qW2, qb2, qW3, qb3, dW, dZ):
    """Per-core input maps. Layout/slicing only — no arithmetic on inputs."""
    f = np.float32
    QSTEPS = (nsteps + NQ - 1) // NQ
    W1row1 = np.concatenate([zW1[1], qW1[1]]).astype(f)  # (128,)
    L1bs = {}
    for g in range(NG):
        L1b = np.zeros((GCH, GCH * 128), f)
        for c in range(GCH):
            L1b[c, c * 128 : (c + 1) * 128] = W1row1
        L1bs[f"L1b{g}"] = L1b
    W1cT = np.ascontiguousarray(np.concatenate([zW1, qW1], axis=1).T).astype(f)
    W2bd = np.zeros((128, 128), f)
    W2bd[0:64, 0:64] = zW2
    W2bd[64:128, 64:128] = qW2
    W3c = np.zeros((128, 4), f)
    W3c[0:64, 0:3] = zW3
    W3c[64:128, 3] = qW3[:, 0]
    b1c = np.concatenate([zb1, qb1]).astype(f).reshape(128, 1)
    b2c = np.concatenate([zb2, qb2]).astype(f).reshape(128, 1)
    b3c = np.concatenate([zb3, qb3]).astype(f).reshape(1, 4)
    trep = np.broadcast_to((np.arange(nsteps) * DT).astype(f), (128, nsteps)).copy()
    ones_col = np.ones((128, 1), f)
    ones_row = np.ones((1, 128), f)
    I128 = np.eye(128, dtype=f)
    y_init = np.broadcast_to(np.asarray(y0, f).reshape(1, 1), (16, 128)).copy()
    Y_init = np.broadcast_to(np.asarray(Y0, f).reshape(1, 1), (128, 16)).copy()

    shared = dict(
        W1cT=W1cT, W2bd=W2bd, W3c=W3c, b1c=b1c, b2c=b2c, b3c=b3c,
        trep=trep, ones_col=ones_col, ones_row=ones_row, I128=I128,
        y_init=y_init, Y_init=Y_init, **L1bs,
    )

    in_maps = []
    for core in range(NCORES):
        o = core * BC
        m = dict(shared)
        for name, arr in (("dWf", dW), ("dZf", dZ)):
            x = np.ascontiguousarray(arr[:nsteps, o : o + BC, :]).astype(f)
            x = x.reshape(nsteps, NCH, 128, 3).transpose(2, 0, 1, 3)
            x = np.ascontiguousarray(x).reshape(128, nsteps * 48)
            for q in range(NQ):
                sl = x[:, q * QSTEPS * 48 : (q + 1) * QSTEPS * 48]
                buf = np.zeros((128, QSTEPS * 48), f)
                buf[:, : sl.shape[1]] = sl
                m[f"{name}{q}"] = buf
        in_maps.append(m)
    return in_maps


def _run(nsteps, inputs, debug=False):
    global LAST_EXEC_NS, LAST_RESULTS
    from concourse import bass_utils

    key = (nsteps, debug)
    if key not in _CACHE:
        _CACHE[key] = _build(nsteps, debug=debug)
    nc = _CACHE[key]

    in_maps = _host_inputs(nsteps, **inputs)
    trace = bool(os.environ.get("BASS_TRACE"))
    kwargs = {}
    if trace:
        import tempfile

        kwargs = dict(trace=True, tmpdir=tempfile.mkdtemp(prefix="bsde_trace_"))
    res = bass_utils.run_bass_kernel_spmd(
        nc, in_maps, core_ids=list(range(NCORES)), **kwargs
    )
    LAST_RESULTS = res
    LAST_EXEC_NS = res.exec_time_ns
    return res


def kernel(**inputs):
    inputs = {k: np.asarray(v, np.float32) for k, v in inputs.items()}
    res = _run(NSTEPS, inputs, debug=False)
    total = np.float32(0.0)
    for core in range(NCORES):
        total += res.results[core]["loss_out"][0, 0]
    return np.array(total, dtype=np.float32)
